# revision 1
# baseline (speedup 1.0000x reference)
"""GAT (graph attention) full-graph kernel for 8 Trainium2 NeuronCores.

Strategy (dst-sharded graph parallel):
  Launch 1 (SPMD, node-sharded): core k projects its 12,500 nodes:
    feat = x @ W (bf16 matmul, fp32 out), el/er = x @ (W @ attn_{l,r}) (fp32).
  Host: assembles the replicated feat table [8*12544, 128]f32, routes every
    edge to the core owning its dst node, buckets edges by
    (dst-tile(128), src-quartile(25088 rows, int16-indexable)), pads buckets
    to 128-edge blocks with a common-across-cores structure, and gathers the
    per-edge el[src]/er[dst] values (launch-1 outputs; pure indexing).
  Launch 2 (SPMD, dst-sharded): per 128-edge block:
    dma_gather feat rows by src (512B rows),
    M_T[e,d] = (dstoff[e]==d) one-hot (DVE compare vs iota),
    ex = exp(leakyrelu(el+er)) (DVE+ACT),
    psum[d, 0:132] += M_T^T @ [feat*ex | ex] (PE, accumulated per dst tile),
    epilogue: out[d] = relu(mean_h(numer_h / s_h) + mean(bias)).

Self-contained: hardcodes problem shapes; host work is integer routing,
gathers of device-computed arrays, and concatenation (all float arithmetic
happens on-device).
"""

import numpy as np
import ml_dtypes

import concourse.bacc as bacc
import concourse.bass as bass
import concourse.mybir as mybir
import concourse.tile as tile
from concourse.bass_utils import run_bass_kernel_spmd
from concourse.bass_interp import get_hw_module
from concourse.library_config import mlp

F32 = mybir.dt.float32
BF16 = mybir.dt.bfloat16
I16 = mybir.dt.int16

# ---- problem constants ----
N = 100000
H = 4
C = 32
E = 1600000
IN = 256
HC = H * C            # 128
NEG = 0.2

NCORES = 8
NPC = N // NCORES     # 12500 nodes per core
TILES = 98            # ceil(12500/128)
NPAD = TILES * 128    # 12544
QROWS = 2 * NPAD      # 25088 table rows per src-quartile (int16-safe)
STILE = 4             # dst tiles per supertile (gather batching)
NSUP = (TILES + STILE - 1) // STILE  # 25

_cache = {}


# --------------------------------------------------------------------------
# Launch 1: sharded projection
# --------------------------------------------------------------------------
def build_launch1(repeat=1):
    nc = bacc.Bacc("TRN2", target_bir_lowering=False, debug=False,
                   num_devices=NCORES)
    xt_d = nc.dram_tensor("xt", [IN, NPAD], F32, kind="ExternalInput")
    w_d = nc.dram_tensor("w", [IN, HC], F32, kind="ExternalInput")
    wt_d = nc.dram_tensor("wt", [HC, IN], F32, kind="ExternalInput")
    alr_d = nc.dram_tensor("alr", [HC, 2 * H], F32, kind="ExternalInput")
    feat_d = nc.dram_tensor("feat", [NPAD, HC], BF16, kind="ExternalOutput")
    el_d = nc.dram_tensor("el", [NPAD, H], F32, kind="ExternalOutput")
    er_d = nc.dram_tensor("er", [NPAD, H], F32, kind="ExternalOutput")

    with tile.TileContext(nc) as tc:
        with (
            tc.tile_pool(name="const", bufs=1) as cp,
            tc.tile_pool(name="work", bufs=3) as wp,
            tc.tile_pool(name="ps", bufs=2, space="PSUM") as ps,
            tc.tile_pool(name="pslr", bufs=2, space="PSUM") as pslr,
            tc.tile_pool(name="psa", bufs=1, space="PSUM") as psa,
        ):
            # a_lr[i, :] = sum_j W[i, j] * A[j, :]  (fp32, 256x8)
            wt_sb = cp.tile([HC, IN], F32)
            nc.sync.dma_start(wt_sb[:], wt_d[:])
            alr_sb = cp.tile([HC, 2 * H], F32)
            nc.sync.dma_start(alr_sb[:], alr_d[:])
            a_sb = []
            for i in range(2):
                pa = psa.tile([128, 2 * H], F32, tag=f"pa{i}")
                nc.tensor.matmul(pa[:], wt_sb[:, i * 128:(i + 1) * 128],
                                 alr_sb[:], start=True, stop=True)
                asb = cp.tile([128, 2 * H], F32, tag=f"a{i}")
                nc.vector.tensor_copy(asb[:], pa[:])
                a_sb.append(asb)

            # W slabs (f32 -> bf16)
            wb = []
            for i in range(2):
                wf = cp.tile([128, HC], F32, tag=f"wf{i}")
                nc.sync.dma_start(wf[:], w_d[i * 128:(i + 1) * 128, :])
                wbi = cp.tile([128, HC], BF16, tag=f"wb{i}")
                nc.scalar.copy(wbi[:], wf[:])
                wb.append(wbi)

            for rep in range(repeat):
              for t in range(TILES):
                cs = slice(t * 128, (t + 1) * 128)
                x0 = wp.tile([128, 128], F32, tag="x0")
                nc.sync.dma_start(x0[:], xt_d[0:128, cs])
                x1 = wp.tile([128, 128], F32, tag="x1")
                nc.sync.dma_start(x1[:], xt_d[128:256, cs])
                xb0 = wp.tile([128, 128], BF16, tag="xb0")
                nc.scalar.copy(xb0[:], x0[:])
                xb1 = wp.tile([128, 128], BF16, tag="xb1")
                nc.scalar.copy(xb1[:], x1[:])

                pf = ps.tile([128, HC], F32, tag="pf")
                nc.tensor.matmul(pf[:], xb0[:], wb[0][:], start=True, stop=False)
                nc.tensor.matmul(pf[:], xb1[:], wb[1][:], start=False, stop=True)

                plr = pslr.tile([128, 2 * H], F32, tag="plr")
                nc.tensor.matmul(plr[:], x0[:], a_sb[0][:], start=True, stop=False)
                nc.tensor.matmul(plr[:], x1[:], a_sb[1][:], start=False, stop=True)

                fsb = wp.tile([128, HC], BF16, tag="fsb")
                nc.scalar.copy(fsb[:], pf[:])
                lr_sb = wp.tile([128, 2 * H], F32, tag="lrsb")
                nc.vector.tensor_copy(lr_sb[:], plr[:])
                nc.sync.dma_start(feat_d[cs, :], fsb[:])
                nc.sync.dma_start(el_d[cs, :], lr_sb[:, 0:H])
                nc.sync.dma_start(er_d[cs, :], lr_sb[:, H:2 * H])
    nc.compile()
    nc.m = get_hw_module(nc.m)
    return nc


# --------------------------------------------------------------------------
# Launch 2: edge phase.  meta = dict with bucket structure (common per core).
# --------------------------------------------------------------------------
def build_launch2(meta, repeat=1):
    nb = meta["nb"]                # [TILES][4] blocks per bucket
    NBS_MAX = meta["nbs_max"]      # max blocks in one supertile
    NBSQ_MAX = meta["nbsq_max"]    # max blocks in one (supertile, quartile)
    BTOT = meta["btot"]            # total blocks
    sup_base = meta["sup_base"]    # block col base per supertile
    TROWS = NCORES * NPAD

    nc = bacc.Bacc("TRN2", target_bir_lowering=False, debug=False,
                   num_devices=NCORES)
    table_d = nc.dram_tensor("table", [TROWS, HC], BF16, kind="ExternalInput")
    idxs_d = nc.dram_tensor("idxs", [128, BTOT * 8], I16, kind="ExternalInput")
    dstoff_d = nc.dram_tensor("dstoff", [128, BTOT], BF16, kind="ExternalInput")
    iota_d = nc.dram_tensor("iota", [128, NBSQ_MAX * 128], BF16,
                            kind="ExternalInput")
    elr_d = nc.dram_tensor("elr", [128, BTOT * 2 * H], F32, kind="ExternalInput")
    bias_d = nc.dram_tensor("bias", [1, HC], F32, kind="ExternalInput")
    out_d = nc.dram_tensor("out", [NPAD, C], F32, kind="ExternalOutput")

    with tile.TileContext(nc) as tc:
        nc.gpsimd.load_library(mlp)
        with (
            tc.tile_pool(name="const", bufs=1) as cp,
            tc.tile_pool(name="sup", bufs=2) as sp,
            tc.tile_pool(name="bk", bufs=2) as bp,
            tc.tile_pool(name="ep", bufs=2) as ep,
            tc.tile_pool(name="pso", bufs=2 * STILE, space="PSUM") as pso,
        ):
            iota = cp.tile([128, NBSQ_MAX * 128], BF16)
            nc.sync.dma_start(iota[:], iota_d[:])

            # bias: mean over heads, broadcast to 128 partitions
            bsb = cp.tile([1, HC], F32)
            nc.sync.dma_start(bsb[:], bias_d[:])
            b01 = cp.tile([1, C], F32)
            nc.vector.tensor_add(b01[:], bsb[:, 0:C], bsb[:, C:2 * C])
            b23 = cp.tile([1, C], F32)
            nc.vector.tensor_add(b23[:], bsb[:, 2 * C:3 * C], bsb[:, 3 * C:4 * C])
            bsum = cp.tile([1, C], F32)
            nc.vector.tensor_add(bsum[:], b01[:], b23[:])
            bmean = cp.tile([1, C], F32)
            nc.vector.tensor_scalar_mul(bmean[:], bsum[:], 0.25)
            ones = cp.tile([1, 128], F32)
            nc.gpsimd.memset(ones[:], 1.0)
            pb = pso.tile([128, HC + H], F32, tag="pout")
            nc.tensor.matmul(pb[:, 0:C], ones[:], bmean[:], start=True, stop=True)
            biasb = cp.tile([128, C], F32)
            nc.vector.tensor_copy(biasb[:], pb[:, 0:C])

            for rep in range(repeat):
              for s in range(NSUP):
                ts = list(range(s * STILE, min((s + 1) * STILE, TILES)))
                nb_sq = [sum(nb[t][q] for t in ts) for q in range(4)]
                nbs = sum(nb_sq)
                if nbs == 0:
                    continue
                cb = sup_base[s]          # global block col base

                idx_sb = sp.tile([128, NBS_MAX * 8], I16, tag="idx")
                nc.sync.dma_start(idx_sb[:, 0:nbs * 8],
                                  idxs_d[:, cb * 8:(cb + nbs) * 8])
                doff_sb = sp.tile([128, NBS_MAX], BF16, tag="doff")
                nc.sync.dma_start(doff_sb[:, 0:nbs], dstoff_d[:, cb:cb + nbs])
                elr_sb = sp.tile([128, NBS_MAX, 2 * H], F32, tag="elr")
                nc.sync.dma_start(
                    elr_sb[:, 0:nbs, :].rearrange("p b h -> p (b h)"),
                    elr_d[:, cb * 2 * H:(cb + nbs) * 2 * H])
                gbuf = sp.tile([128, NBS_MAX, HC], BF16, tag="gbuf")

                qb = [0, 0, 0, 0]
                acc = 0
                for q in range(4):
                    qb[q] = acc
                    acc += nb_sq[q]

                pouts = {}
                for t in ts:
                    pt_ = pso.tile([128, HC + H], F32, tag="pout", name=f"pout{t}")
                    pouts[t] = pt_
                done_b = {t: 0 for t in ts}
                total_b = {t: sum(nb[t]) for t in ts}

                for q in range(4):
                    nq = nb_sq[q]
                    if nq == 0:
                        continue
                    L = nq * 128
                    nc.gpsimd.dma_gather(
                        gbuf[:, qb[q]:qb[q] + nq, :],
                        table_d[q * QROWS:(q + 1) * QROWS, :],
                        idx_sb[:, qb[q] * 8:(qb[q] + nq) * 8],
                        L, L, HC, single_packet=False,
                    )
                    gq = gbuf[:, qb[q]:qb[q] + nq, :]
                    # ---- batched per (s, q) ----
                    # one-hot M_T [128e, nq, 128d]
                    mt = bp.tile([128, NBSQ_MAX, 128], BF16, tag="mt")
                    doff_bc = bass.AP(
                        doff_sb.tensor, doff_sb[:, qb[q]:qb[q] + nq].offset,
                        [doff_sb[:].ap[0], [1, nq], [0, 128]])
                    nc.vector.tensor_tensor(
                        out=mt[:, 0:nq, :], in0=doff_bc,
                        in1=iota[:, 0:nq * 128].rearrange(
                            "p (b d) -> p b d", d=128),
                        op=mybir.AluOpType.is_equal)
                    # e2 = leakyrelu(el + er)
                    e_sb = bp.tile([128, NBSQ_MAX * H], F32, tag="e")
                    nc.vector.tensor_tensor(
                        out=e_sb[:, 0:nq * H].rearrange(
                            "p (b h) -> p b h", h=H),
                        in0=elr_sb[:, qb[q]:qb[q] + nq, 0:H],
                        in1=elr_sb[:, qb[q]:qb[q] + nq, H:2 * H],
                        op=mybir.AluOpType.add)
                    t1 = bp.tile([128, NBSQ_MAX * H], F32, tag="t1")
                    nc.vector.tensor_scalar_mul(
                        t1[:, 0:nq * H], e_sb[:, 0:nq * H], NEG)
                    e2 = bp.tile([128, NBSQ_MAX * H], F32, tag="e2")
                    nc.vector.tensor_tensor(
                        out=e2[:, 0:nq * H], in0=e_sb[:, 0:nq * H],
                        in1=t1[:, 0:nq * H], op=mybir.AluOpType.max)
                    # exd = exp(e2) broadcast-expanded to [128, nq, H*C] (ACT)
                    exd = bp.tile([128, NBSQ_MAX, HC], BF16, tag="exd")
                    e2_bc = bass.AP(
                        e2.tensor, e2[:].offset,
                        [e2[:].ap[0], [H, nq], [1, H], [0, C]])
                    exd4 = bass.AP(
                        exd.tensor, exd[:].offset,
                        [exd[:].ap[0], [HC, nq], [C, H], [1, C]])
                    nc.scalar.activation(exd4, e2_bc,
                                         mybir.ActivationFunctionType.Exp)
                    # rhs = [feat * exd | ex]  (all-bf16 packed -> 2x mode)
                    rhs = bp.tile([128, NBSQ_MAX, HC + H], BF16, tag="rhs")
                    nc.vector.tensor_tensor(
                        out=rhs[:, 0:nq, 0:HC], in0=gq, in1=exd[:, 0:nq, :],
                        op=mybir.AluOpType.mult)
                    ex_cols = bass.AP(
                        exd.tensor, exd[:].offset,
                        [exd[:].ap[0], [HC, nq], [C, H]])
                    nc.vector.tensor_copy(rhs[:, 0:nq, HC:HC + H], ex_cols)
                    # aggregate into per-tile psums
                    for ti, t in enumerate(ts):
                        cnt = nb[t][q]
                        if cnt == 0:
                            continue
                        off = qb[q] + sum(nb[t2][q] for t2 in ts[:ti]) - qb[q]
                        for j in range(cnt):
                            jb = off + j
                            nc.tensor.matmul(
                                pouts[t][:], mt[:, jb, :],
                                rhs[:, jb, :],
                                start=(done_b[t] == 0),
                                stop=(done_b[t] == total_b[t] - 1),
                                skip_group_check=True)
                            done_b[t] += 1

                # ---- epilogues ----
                for t in ts:
                    pout = pouts[t]
                    s4 = ep.tile([128, H], F32, tag="s4")
                    nc.vector.tensor_scalar(
                        out=s4[:], in0=pout[:, HC:HC + H], scalar1=4.0,
                        scalar2=1e-20, op0=mybir.AluOpType.mult,
                        op1=mybir.AluOpType.add)
                    srec = ep.tile([128, H], F32, tag="srec")
                    nc.vector.reciprocal_approx_fast(srec[:], s4[:])
                    scaled = ep.tile([128, H, C], F32, tag="scaled")
                    srec_bc = bass.AP(srec.tensor, srec[:].offset,
                                      [srec[:].ap[0], [1, H], [0, C]])
                    nc.vector.tensor_tensor(
                        out=scaled[:],
                        in0=pout[:, 0:HC].rearrange("p (h c) -> p h c", c=C),
                        in1=srec_bc, op=mybir.AluOpType.mult)
                    h01 = ep.tile([128, C], F32, tag="h01")
                    nc.vector.tensor_add(h01[:], scaled[:, 0, :], scaled[:, 1, :])
                    h23 = ep.tile([128, C], F32, tag="h23")
                    nc.vector.tensor_add(h23[:], scaled[:, 2, :], scaled[:, 3, :])
                    hs = ep.tile([128, C], F32, tag="hs")
                    nc.vector.tensor_add(hs[:], h01[:], h23[:])
                    hb = ep.tile([128, C], F32, tag="hb")
                    nc.vector.tensor_add(hb[:], hs[:], biasb[:])
                    outt = ep.tile([128, C], F32, tag="outt")
                    nc.vector.tensor_scalar_max(outt[:], hb[:], 0.0)
                    nc.sync.dma_start(out_d[t * 128:(t + 1) * 128, :], outt[:])
    nc.compile()
    nc.m = get_hw_module(nc.m)
    return nc


# --------------------------------------------------------------------------
# Host-side routing
# --------------------------------------------------------------------------
def balance_tiles(owner, dloc, q):
    """Assign each core's nodes to dst tiles so that per-(tile, src-quartile)
    edge counts stay <= 512 (4 blocks of 128) where possible.  Returns
    perm[NCORES, NPC]: original local node -> tile*128 + slot."""
    target = 4 * 128
    perm = np.zeros((NCORES, NPC), np.int64)
    for k in range(NCORES):
        m = owner == k
        dv = np.zeros((NPC, 4), np.int64)
        np.add.at(dv, (dloc[m], q[m]), 1)
        order = np.argsort(-dv.sum(1), kind="stable")
        L = np.zeros((TILES, 4), np.int64)
        cnt = np.zeros(TILES, np.int64)
        cap = np.full(TILES, 128, np.int64)
        cap[TILES - 1] = NPC - (TILES - 1) * 128
        assign = np.zeros(NPC, np.int64)
        for n in order:
            d = dv[n]
            pen = np.maximum(L + d - target, 0).sum(1).astype(np.float64)
            pen += (L + d).max(1) * 1e-6      # tie-break: keep tiles level
            pen[cnt >= cap] = np.inf
            t = int(np.argmin(pen))
            L[t] += d
            assign[n] = t
            cnt[t] += 1

        # swap refinement: move overflow out of >target buckets
        tile_nodes = [np.where(assign == t)[0] for t in range(TILES)]
        for _ in range(6):
            over = np.maximum(L - target, 0)
            if over.sum() == 0:
                break
            improved = False
            for t in np.argsort(-over.sum(1)):
                if over[t].sum() == 0:
                    continue
                nt = tile_nodes[t]
                dvt = dv[nt]                       # [nt, 4]
                pen_t0 = np.maximum(L[t] - target, 0).sum()
                # candidate partner tiles: least loaded in the worst quartile
                qw = int(np.argmax(over[t]))
                cand = np.argsort(L[:, qw])[:8]
                best = None
                for t2 in cand:
                    if t2 == t:
                        continue
                    nt2 = tile_nodes[t2]
                    dvt2 = dv[nt2]                 # [m, 4]
                    pen_20 = np.maximum(L[t2] - target, 0).sum()
                    # pairwise swap deltas: d = dvt2[m] - dvt[n]
                    dd = dvt2[None, :, :] - dvt[:, None, :]   # [n, m, 4]
                    p1 = np.maximum(L[t] + dd - target, 0).sum(2)
                    p2 = np.maximum(L[t2] - dd - target, 0).sum(2)
                    gain = (pen_t0 + pen_20) - (p1 + p2)
                    i, j = np.unravel_index(np.argmax(gain), gain.shape)
                    if gain[i, j] > 0 and (best is None or gain[i, j] > best[0]):
                        best = (gain[i, j], int(t2), int(i), int(j))
                if best is not None:
                    _, t2, i, j = best
                    n1 = tile_nodes[t][i]
                    n2 = tile_nodes[t2][j]
                    L[t] += dv[n2] - dv[n1]
                    L[t2] += dv[n1] - dv[n2]
                    tile_nodes[t][i] = n2
                    tile_nodes[t2][j] = n1
                    assign[n1], assign[n2] = t2, t
                    improved = True
            if not improved:
                break

        slots = np.zeros(TILES, np.int64)
        for t in range(TILES):
            nt = tile_nodes[t]
            perm[k, nt] = t * 128 + np.arange(len(nt))
    return perm


def route_edges(src, dst):
    """Bucket edges by (owner core, dst tile, src quartile); pad to common
    128-edge blocks.  Returns meta + per-core index arrays."""
    src = src.astype(np.int64)
    dst = dst.astype(np.int64)
    owner = dst // NPC
    dloc = dst - owner * NPC
    row = (src // NPC) * NPAD + (src % NPC)
    q = row // QROWS
    idx16 = (row - q * QROWS).astype(np.int16)
    drow = owner * NPAD + dloc          # padded row of dst node (launch1 order)

    perm = balance_tiles(owner, dloc, q)
    slot = perm[owner, dloc]            # balanced slot of each edge's dst
    t_id = slot >> 7
    doff = (slot & 127).astype(np.float32)

    # bucket key: (owner, supertile, quartile, tile)
    sidx = t_id // STILE
    key = ((owner * NSUP + sidx) * 4 + q) * TILES + t_id
    order = np.argsort(key, kind="stable")
    key_s = key[order]
    idx16_s = idx16[order]
    doff_s = doff[order]
    row_s = row[order]
    drow_s = drow[order]

    cnt = np.bincount((owner * TILES + t_id) * 4 + q,
                      minlength=NCORES * TILES * 4).reshape(NCORES, TILES, 4)
    nb = np.ceil(cnt.max(axis=0) / 128.0).astype(np.int64)   # [TILES, 4]
    nbmax = int(nb.max())
    btot = int(nb.sum())
    epad = btot * 128

    boff = np.zeros((TILES, 4), np.int64)
    sup_base = []
    nbs_max = 0
    nbsq_max = 0
    cur = 0
    for s in range(NSUP):
        sup_base.append(cur)
        ts = range(s * STILE, min((s + 1) * STILE, TILES))
        for qq in range(4):
            q0 = cur
            for t in ts:
                boff[t, qq] = cur
                cur += nb[t, qq]
            nbsq_max = max(nbsq_max, cur - q0)
        nbs_max = max(nbs_max, cur - sup_base[-1])
    assert cur == btot

    idx_all = np.zeros((NCORES, epad), np.int16)
    dst_all = np.full((NCORES, epad), -1.0, np.float32)
    srow_all = np.zeros((NCORES, epad), np.int64)   # padded table row of src
    drow_all = np.zeros((NCORES, epad), np.int64)   # padded table row of dst

    core_of = key_s // (NSUP * 4 * TILES)
    core_starts = np.searchsorted(core_of, np.arange(NCORES + 1))
    for k in range(NCORES):
        a, b = core_starts[k], core_starts[k + 1]
        kk = key_s[a:b]
        ub, inv, ucnt = np.unique(kk, return_inverse=True, return_counts=True)
        starts = np.zeros(len(ub), np.int64)
        starts[1:] = np.cumsum(ucnt)[:-1]
        rank = np.arange(b - a) - starts[inv]
        ut = ub % TILES
        uq = (ub // TILES) % 4
        base = boff[ut, uq] * 128
        pos = base[inv] + rank
        idx_all[k, pos] = idx16_s[a:b]
        dst_all[k, pos] = doff_s[a:b]
        srow_all[k, pos] = row_s[a:b]
        drow_all[k, pos] = drow_s[a:b]
        # pads: idx16 stays 0 (valid row of the quartile); srow/drow 0 (finite)

    # idx pads must be valid *within their quartile*: idx 0 maps to row
    # q*QROWS which exists for every quartile -> fine.

    # wrap idxs for dma_gather: within each (s, q) gather segment,
    # idx position i -> partition i%16 (replicated x8), col i//16
    idxs_host = np.zeros((NCORES, 128, btot * 8), np.int16)
    for s in range(NSUP):
        ts = range(s * STILE, min((s + 1) * STILE, TILES))
        for qq in range(4):
            tl = [t for t in ts if nb[t, qq] > 0]
            if not tl:
                continue
            a = boff[tl[0], qq] * 128
            L = int(sum(nb[t, qq] for t in tl)) * 128
            seg = idx_all[:, a:a + L]                       # [NCORES, L]
            wrap = seg.reshape(NCORES, L // 16, 16).transpose(0, 2, 1)
            cb = a // 16                                    # = block*8
            idxs_host[:, :, cb:cb + L // 16] = np.tile(wrap, (1, 8, 1))

    dst_host = dst_all.reshape(NCORES, btot, 128).transpose(0, 2, 1)
    dst_host = np.ascontiguousarray(dst_host.astype(ml_dtypes.bfloat16))

    meta = {
        "nb": nb.tolist(),
        "nbmax": nbmax,
        "nbs_max": int(nbs_max),
        "nbsq_max": int(nbsq_max),
        "btot": btot,
        "sup_base": sup_base,
    }
    return meta, idxs_host, dst_host, srow_all, drow_all, perm


# --------------------------------------------------------------------------
def kernel(x, src, dst, W, attn_l, attn_r, bias):
    x = np.asarray(x, dtype=np.float32)
    src = np.asarray(src)
    dst = np.asarray(dst)
    W = np.asarray(W, dtype=np.float32)
    attn_l = np.asarray(attn_l, dtype=np.float32)
    attn_r = np.asarray(attn_r, dtype=np.float32)
    bias = np.asarray(bias, dtype=np.float32)

    meta, idxs_host, dst_host, srow_all, drow_all, perm = route_edges(src, dst)

    # ---- launch 1 ----
    if "l1" not in _cache:
        _cache["l1"] = build_launch1()
    nc1 = _cache["l1"]

    xt = np.ascontiguousarray(x.T)                     # [256, 100000]
    alr = np.zeros((HC, 2 * H), np.float32)            # block-diag attn layout
    for h in range(H):
        alr[h * C:(h + 1) * C, h] = attn_l[h]
        alr[h * C:(h + 1) * C, H + h] = attn_r[h]
    wt = np.ascontiguousarray(W.T)                     # [128, 256]

    in1 = []
    for k in range(NCORES):
        xtk = np.zeros((IN, NPAD), np.float32)
        xtk[:, :NPC] = xt[:, k * NPC:(k + 1) * NPC]
        in1.append({"xt": xtk, "w": W, "wt": wt, "alr": alr})
    res1 = run_bass_kernel_spmd(nc1, in1, list(range(NCORES)))

    table = np.concatenate([res1.results[k]["feat"] for k in range(NCORES)])
    el_g = np.concatenate([res1.results[k]["el"] for k in range(NCORES)])
    er_g = np.concatenate([res1.results[k]["er"] for k in range(NCORES)])

    # ---- launch 2 inputs ----
    key2 = (meta["btot"], meta["nbmax"], meta["nbs_max"], meta["nbsq_max"],
            tuple(tuple(r) for r in meta["nb"]))
    if ("l2", key2) not in _cache:
        _cache[("l2", key2)] = build_launch2(meta)
    nc2 = _cache[("l2", key2)]

    iota = np.tile(np.arange(128, dtype=np.float32),
                   meta["nbsq_max"]).reshape(1, -1).repeat(128, 0)
    iota = np.ascontiguousarray(iota.astype(ml_dtypes.bfloat16))
    bias2 = bias.reshape(1, HC)
    btot = meta["btot"]

    in2 = []
    for k in range(NCORES):
        # per-edge el[src], er[dst] (block-partition layout [128, btot, 8])
        elr = np.empty((btot, 128, 2 * H), np.float32)
        elr[:, :, 0:H] = el_g[srow_all[k]].reshape(btot, 128, H)
        elr[:, :, H:2 * H] = er_g[drow_all[k]].reshape(btot, 128, H)
        elr = np.ascontiguousarray(
            elr.transpose(1, 0, 2).reshape(128, btot * 2 * H))
        in2.append({
            "table": table,
            "idxs": idxs_host[k],
            "dstoff": dst_host[k],
            "iota": iota,
            "elr": elr,
            "bias": bias2,
        })
    res2 = run_bass_kernel_spmd(nc2, in2, list(range(NCORES)))

    out = np.concatenate(
        [res2.results[k]["out"][perm[k]] for k in range(NCORES)])
    return out.astype(np.float32)

